# revision 9
# baseline (speedup 1.0000x reference)
# Distributed causal self-attention for 8 Trainium2 NeuronCores.
#
# Problem: B=2, T=2048, C=768, H=12 heads, D=64. y = proj(attn(qkv(x))).
#
# Sharding: 2 (batch) x 4 (head-groups of 3 heads). Core c handles batch
# c//4 and heads (c%4)*3 .. +3. Each core computes its slice of the QKV
# projection, full attention for its 3 heads, and a partial output
# projection y_part = O_heads @ Wp_slice.T. Host sums the 4 partials per
# batch and adds b_proj.
#
# Device-side layout avoids all transposes:
#   qT,kT [d, t]   <- Wqk stationary, xT moving       (per head: 64 rows)
#   sT    [tk, tq] <- k-slice stationary, qT moving   (causal: skip tq < tk_tile)
#   exp   via ScalarE (no max subtraction: |s| <= ~40, exp fits f32)
#   OT+rowsum [65, tq] <- [v | ones] stationary, exp(sT) moving (f32 accum)
#   normalize: reciprocal of row 64, K=1 ones-matmul broadcast, DVE mult
#   y     [t, c2]  <- OT stationary (K=192 over head dims), WpT moving
# Matmul operands bf16, accumulation f32.

import numpy as np

B, T, C, H, D = 2, 2048, 768, 12, 64
HPG = 3                      # heads per group
G = 4                        # head groups
CPG = HPG * D                # 192 channels per group
KT = C // 128                # 6 contraction tiles for projections
NT = T // 128                # 16 seq tiles
NCH = T // 512               # 4 column chunks of 512
SCALE = float(1.0 / np.sqrt(2.0))   # 1/sqrt(B) (faithful to reference)

_CACHE = {}


def _build_module():
    import concourse.bass as bass
    import concourse.tile as tile
    import concourse.mybir as mybir
    from concourse.bacc import Bacc
    from contextlib import ExitStack

    f32 = mybir.dt.float32
    bf16 = mybir.dt.bfloat16
    AF = mybir.ActivationFunctionType

    # Bacc (not plain Bass): its compile() legalizes sync waits to the TRN2
    # hardware limit of one wait per instruction (generate_event_semaphores).
    nc = Bacc()

    xt_d = nc.dram_tensor("xt", [C, T], bf16, kind="ExternalInput")
    wqkt_d = nc.dram_tensor("wqkt", [C, 2 * CPG], bf16, kind="ExternalInput")
    wvt_d = nc.dram_tensor("wvt", [C, CPG], bf16, kind="ExternalInput")
    bqk_d = nc.dram_tensor("bqk", [128, 4], f32, kind="ExternalInput")
    bv_d = nc.dram_tensor("bv", [128, CPG], f32, kind="ExternalInput")
    wpt_d = nc.dram_tensor("wpt", [CPG, C], bf16, kind="ExternalInput")
    mask_d = nc.dram_tensor("mask", [128, 128], bf16, kind="ExternalInput")
    y_d = nc.dram_tensor("y", [T, C], f32, kind="ExternalOutput")

    with tile.TileContext(nc) as tc, ExitStack() as ctx:
        sb = ctx.enter_context(tc.tile_pool(name="sb", bufs=1))
        ps = ctx.enter_context(tc.tile_pool(name="ps", bufs=1, space="PSUM"))

        # ---- constants and weights into SBUF ----
        xt_sb = []
        for k in range(KT):
            t_ = sb.tile([128, T], bf16, tag=f"xt{k}", name=f"xt{k}")
            nc.sync.dma_start(t_[:, :], xt_d[k * 128:(k + 1) * 128, :])
            xt_sb.append(t_)
        wqkt_sb = []
        for k in range(KT):
            t_ = sb.tile([128, 2 * CPG], bf16, tag=f"wqk{k}", name=f"wqk{k}")
            nc.sync.dma_start(t_[:, :], wqkt_d[k * 128:(k + 1) * 128, :])
            wqkt_sb.append(t_)
        wvt_sb = []
        for k in range(KT):
            t_ = sb.tile([128, CPG], bf16, tag=f"wv{k}", name=f"wv{k}")
            nc.sync.dma_start(t_[:, :], wvt_d[k * 128:(k + 1) * 128, :])
            wvt_sb.append(t_)
        bqk_sb = sb.tile([128, 4], f32, tag="bqk", name="bqk")
        nc.sync.dma_start(bqk_sb[:, :], bqk_d[:, :])
        bv_sb = sb.tile([128, CPG], f32, tag="bv", name="bv")
        nc.sync.dma_start(bv_sb[:, :], bv_d[:, :])
        mask_sb = sb.tile([128, 128], bf16, tag="mask", name="mask")
        nc.sync.dma_start(mask_sb[:, :], mask_d[:, :])
        wpt0_sb = sb.tile([128, C], bf16, tag="wpt0", name="wpt0")
        nc.sync.dma_start(wpt0_sb[:, :], wpt_d[0:128, :])
        wpt1_sb = sb.tile([64, C], bf16, tag="wpt1", name="wpt1")
        nc.sync.dma_start(wpt1_sb[:, :], wpt_d[128:CPG, :])
        ones_sb = sb.tile([1, 64], bf16, tag="ones", name="ones")
        nc.vector.memset(ones_sb[:, :], 1.0)

        # ---- QKV projection (q,k transposed; v natural) ----
        qt0 = sb.tile([128, T], bf16, tag="qt0", name="qt0")
        qt1 = sb.tile([64, T], bf16, tag="qt1", name="qt1")
        kt0 = sb.tile([128, T], bf16, tag="kt0", name="kt0")
        kt1 = sb.tile([64, T], bf16, tag="kt1", name="kt1")
        mtiles = [(qt0, 128, 0), (qt1, 64, 128), (kt0, 128, 192), (kt1, 64, 320)]
        for m, (dst, rows, c0) in enumerate(mtiles):
            for c in range(NCH):
                pq = ps.tile([128, 512], f32, tag="mm", bufs=4, name=f"pq{m}_{c}")
                for k in range(KT):
                    nc.tensor.matmul(
                        pq[0:rows, :],
                        lhsT=wqkt_sb[k][:, c0:c0 + rows],
                        rhs=xt_sb[k][:, c * 512:(c + 1) * 512],
                        start=(k == 0), stop=(k == KT - 1),
                    )
                nc.scalar.activation(
                    dst[:, c * 512:(c + 1) * 512], pq[0:rows, :],
                    AF.Identity, bias=bqk_sb[0:rows, m:m + 1],
                )

        v_sb = []
        for t in range(NT):
            pv = ps.tile([128, 512], f32, tag="mm", bufs=4, name=f"pv{t}")
            for k in range(KT):
                nc.tensor.matmul(
                    pv[:, 0:CPG],
                    lhsT=xt_sb[k][:, t * 128:(t + 1) * 128],
                    rhs=wvt_sb[k][:, :],
                    start=(k == 0), stop=(k == KT - 1),
                )
            vt = sb.tile([128, HPG * 65], bf16, tag=f"v{t}", name=f"v{t}")
            vt3 = vt.rearrange("p (h u) -> p h u", u=65)
            nc.vector.memset(vt3[:, :, 64:65], 1.0)
            nc.vector.tensor_add(
                vt3[:, :, 0:64],
                pv[:, 0:CPG].rearrange("p (h d) -> p h d", d=64),
                bv_sb[:, :].rearrange("p (h d) -> p h d", d=64),
            )
            v_sb.append(vt)

        # ---- attention, one head at a time ----
        q_slices = [(qt0, 0), (qt0, 64), (qt1, 0)]
        k_slices = [(kt0, 0), (kt0, 64), (kt1, 0)]
        pt0 = sb.tile([128, T], bf16, tag="pt0", name="pt0")
        pt1 = sb.tile([64, T], bf16, tag="pt1", name="pt1")
        p_slices = [(pt0, 0), (pt0, 64), (pt1, 0)]

        for h in range(HPG):
            qa, qo = q_slices[h]
            ka, ko = k_slices[h]
            qv = qa[qo:qo + 64, :]
            kv = ka[ko:ko + 64, :]
            ot = ps.tile([65, T], f32, tag="ot", bufs=1, name=f"ot{h}")
            ex_tiles = [None] * NT

            def emit_s(i):
                ex = sb.tile([128, T], bf16, tag="ex", bufs=4, name=f"ex{h}_{i}")
                for j in range(i // 4, NCH):
                    cs = max(j * 512, i * 128)
                    n = (j + 1) * 512 - cs
                    sp = ps.tile([128, 512], f32, tag="mm", bufs=4,
                                 name=f"sp{h}_{i}_{j}")
                    nc.tensor.matmul(
                        sp[:, 0:n],
                        lhsT=kv[:, i * 128:(i + 1) * 128],
                        rhs=qv[:, cs:cs + n],
                        start=True, stop=True,
                    )
                    nc.scalar.activation(ex[:, cs:cs + n], sp[:, 0:n],
                                         AF.Exp, scale=SCALE)
                nc.vector.tensor_mul(
                    ex[:, i * 128:(i + 1) * 128],
                    ex[:, i * 128:(i + 1) * 128],
                    mask_sb[:, :],
                )
                ex_tiles[i] = ex

            def emit_o(i):
                ex = ex_tiles[i]
                for j in range(i // 4, NCH):
                    cs = max(j * 512, i * 128)
                    n = (j + 1) * 512 - cs
                    nc.tensor.matmul(
                        ot[:, cs:cs + n],
                        lhsT=v_sb[i][:, h * 65:(h + 1) * 65],
                        rhs=ex[:, cs:cs + n],
                        start=(i == 0), stop=(i == 4 * j + 3),
                    )

            # software pipeline: S(i) runs 2 iterations ahead of O(i) so the
            # ScalarE exp never stalls the PE.
            for i in range(NT):
                emit_s(i)
                if i >= 2:
                    emit_o(i - 2)
            emit_o(NT - 2)
            emit_o(NT - 1)

            # normalize: rows 0:64 of ot divided by rowsum in row 64.
            # recip -> bf16 -> K=1 ones-matmul broadcast into PSUM; evict the
            # unnormalized O to SBUF on ScalarE so the DVE multiply reads only
            # one PSUM operand (walrus NCC_IBVF027).
            rc = sb.tile([1, T], f32, tag="rc", name=f"rc{h}")
            nc.vector.reciprocal(rc[:, :], ot[64:65, :])
            rcb = sb.tile([1, T], bf16, tag="rcb", name=f"rcb{h}")
            nc.vector.tensor_copy(rcb[:, :], rc[:, :])
            osb = sb.tile([64, T], bf16, tag="osb", name=f"osb{h}")
            nc.scalar.activation(osb[:, :], ot[0:64, :], AF.Identity)
            pdst, po = p_slices[h]
            for j in range(NCH):
                bp = ps.tile([64, 512], f32, tag="mm", bufs=4, name=f"bp{h}_{j}")
                nc.tensor.matmul(bp[:, :], lhsT=ones_sb[:, :],
                                 rhs=rcb[:, j * 512:(j + 1) * 512],
                                 start=True, stop=True)
                nc.vector.tensor_mul(
                    pdst[po:po + 64, j * 512:(j + 1) * 512],
                    osb[:, j * 512:(j + 1) * 512],
                    bp[:, :],
                )

        # ---- output projection (partial over this core's 192 channels) ----
        for t in range(NT):
            ysb = sb.tile([128, C], f32, tag="ysb", bufs=2, name=f"ysb{t}")
            for n0, nn in ((0, 512), (512, 256)):
                yp = ps.tile([128, 512], f32, tag="mm", bufs=4, name=f"yp{t}_{n0}")
                nc.tensor.matmul(yp[:, 0:nn],
                                 lhsT=pt0[:, t * 128:(t + 1) * 128],
                                 rhs=wpt0_sb[:, n0:n0 + nn],
                                 start=True, stop=False)
                nc.tensor.matmul(yp[:, 0:nn],
                                 lhsT=pt1[:, t * 128:(t + 1) * 128],
                                 rhs=wpt1_sb[:, n0:n0 + nn],
                                 start=False, stop=True)
                nc.scalar.activation(ysb[:, n0:n0 + nn], yp[:, 0:nn], AF.Identity)
            nc.sync.dma_start(y_d[t * 128:(t + 1) * 128, :], ysb[:, :])

    nc.finalize()
    return nc


def _get_module():
    if "nc" not in _CACHE:
        _CACHE["nc"] = _build_module()
    return _CACHE["nc"]


def make_in_maps(x, w_attn, b_attn, w_proj):
    """Host-side sharding: per-core input dicts (8 cores)."""
    import ml_dtypes
    bf16 = ml_dtypes.bfloat16
    x = np.asarray(x, dtype=np.float32)
    w_attn = np.asarray(w_attn, dtype=np.float32)
    b_attn = np.asarray(b_attn, dtype=np.float32)
    w_proj = np.asarray(w_proj, dtype=np.float32)

    xts = [np.ascontiguousarray(x[b].T).astype(bf16) for b in range(B)]
    mask = np.triu(np.ones((128, 128), np.float32)).astype(bf16)

    in_maps = []
    for c in range(8):
        b = c // G
        hg = c % G
        sl = slice(CPG * hg, CPG * (hg + 1))
        wq = w_attn[0:C][sl]
        wk = w_attn[C:2 * C][sl]
        wv = w_attn[2 * C:3 * C][sl]
        wqkt = np.ascontiguousarray(
            np.concatenate([wq, wk], axis=0).T).astype(bf16)      # [768, 384]
        wvt = np.ascontiguousarray(wv.T).astype(bf16)             # [768, 192]
        bq = b_attn[0:C][sl]
        bk = b_attn[C:2 * C][sl]
        bv = b_attn[2 * C:3 * C][sl]
        bqk = np.zeros((128, 4), np.float32)
        bqk[:, 0] = bq[0:128]
        bqk[0:64, 1] = bq[128:192]
        bqk[:, 2] = bk[0:128]
        bqk[0:64, 3] = bk[128:192]
        bvb = np.ascontiguousarray(
            np.broadcast_to(bv, (128, CPG))).astype(np.float32)   # [128, 192]
        wpt = np.ascontiguousarray(w_proj[:, sl].T).astype(bf16)  # [192, 768]
        in_maps.append({
            "xt": xts[b],
            "wqkt": wqkt,
            "wvt": wvt,
            "bqk": bqk,
            "bv": bvb,
            "wpt": wpt,
            "mask": mask,
        })
    return in_maps


def gather(results, b_proj):
    """Sum the 4 head-group partials per batch, add bias."""
    b_proj = np.asarray(b_proj, dtype=np.float32)
    y = np.zeros((B, T, C), np.float32)
    for c in range(8):
        y[c // G] += np.asarray(results[c]["y"], dtype=np.float32)
    y += b_proj
    return y


def run(x, w_attn, b_attn, w_proj, b_proj, trace=False, **kw):
    from concourse.bass_utils import run_bass_kernel_spmd
    nc = _get_module()
    in_maps = make_in_maps(x, w_attn, b_attn, w_proj)
    res = run_bass_kernel_spmd(nc, in_maps, list(range(8)), trace=trace, **kw)
    return gather(res.results, b_proj), res


def kernel(x, w_attn, b_attn, w_proj, b_proj):
    y, _ = run(x, w_attn, b_attn, w_proj, b_proj)
    return y


# revision 10
# speedup vs baseline: 1.1532x; 1.1532x over previous
# Distributed causal self-attention for 8 Trainium2 NeuronCores.
#
# Problem: B=2, T=2048, C=768, H=12 heads, D=64. y = proj(attn(qkv(x))).
#
# Sharding: 2 (batch) x 4 (head-groups of 3 heads). Core c handles batch
# c//4 and heads (c%4)*3 .. +3. Each core computes its slice of the QKV
# projection, full attention for its 3 heads, and a partial output
# projection y_part = O_heads @ Wp_slice.T. Host sums the 4 partials per
# batch and adds b_proj.
#
# Device-side layout avoids all transposes:
#   qT,kT [d, t]   <- Wqk stationary, xT moving       (per head: 64 rows)
#   sT    [tk, tq] <- k-slice stationary, qT moving   (causal: skip tq < tk_tile)
#   exp   via ScalarE (no max subtraction: |s| <= ~40, exp fits f32)
#   OT+rowsum [65, tq] <- [v | ones] stationary, exp(sT) moving (f32 accum)
#   normalize: ones-matmul broadcast of rowsum, reciprocal_approx_fast, mult
#   y     [t, c2]  <- OT stationary (K=192 over head dims), WpT moving
# Matmul operands bf16, accumulation f32. ScalarE does ONLY exp (it is the
# bottleneck engine); all PSUM evictions run on DVE, causal masks on GPSIMD.
# Attention runs in two tq-passes of 1024 so the OT accumulator takes 2 PSUM
# banks, leaving 6 banks of [128,1024] working tiles.

import numpy as np

B, T, C, H, D = 2, 2048, 768, 12, 64
HPG = 3                      # heads per group
G = 4                        # head groups
CPG = HPG * D                # 192 channels per group
KT = C // 128                # 6 contraction tiles for projections
NT = T // 128                # 16 seq tiles
PW = 1024                    # tq pass width
SCALE = float(1.0 / np.sqrt(2.0))   # 1/sqrt(B) (faithful to reference)

_CACHE = {}


def _build_module():
    import concourse.bass as bass
    import concourse.tile as tile
    import concourse.mybir as mybir
    from concourse.bacc import Bacc
    from contextlib import ExitStack

    f32 = mybir.dt.float32
    bf16 = mybir.dt.bfloat16
    AF = mybir.ActivationFunctionType

    # Bacc (not plain Bass): its compile() legalizes sync waits to the TRN2
    # hardware limit of one wait per instruction (generate_event_semaphores).
    nc = Bacc()

    xt_d = nc.dram_tensor("xt", [C, T], bf16, kind="ExternalInput")
    wqkt_d = nc.dram_tensor("wqkt", [C, 2 * CPG], bf16, kind="ExternalInput")
    wvt_d = nc.dram_tensor("wvt", [C, CPG], bf16, kind="ExternalInput")
    bqk_d = nc.dram_tensor("bqk", [128, 4], f32, kind="ExternalInput")
    bv_d = nc.dram_tensor("bv", [128, CPG], f32, kind="ExternalInput")
    wpt_d = nc.dram_tensor("wpt", [CPG, C], bf16, kind="ExternalInput")
    mask_d = nc.dram_tensor("mask", [128, 128], bf16, kind="ExternalInput")
    y_d = nc.dram_tensor("y", [T, C], f32, kind="ExternalOutput")

    with tile.TileContext(nc) as tc, ExitStack() as ctx:
        sb = ctx.enter_context(tc.tile_pool(name="sb", bufs=1))
        ps = ctx.enter_context(tc.tile_pool(name="ps", bufs=1, space="PSUM"))

        def mm_tile(name):
            return ps.tile([128, 1024], f32, tag="mm", bufs=3, name=name)

        # ---- constants and weights into SBUF ----
        xt_sb = []
        for k in range(KT):
            t_ = sb.tile([128, T], bf16, tag=f"xt{k}", name=f"xt{k}")
            nc.sync.dma_start(t_[:, :], xt_d[k * 128:(k + 1) * 128, :])
            xt_sb.append(t_)
        wqkt_sb = []
        for k in range(KT):
            t_ = sb.tile([128, 2 * CPG], bf16, tag=f"wqk{k}", name=f"wqk{k}")
            nc.sync.dma_start(t_[:, :], wqkt_d[k * 128:(k + 1) * 128, :])
            wqkt_sb.append(t_)
        wvt_sb = []
        for k in range(KT):
            t_ = sb.tile([128, CPG], bf16, tag=f"wv{k}", name=f"wv{k}")
            nc.sync.dma_start(t_[:, :], wvt_d[k * 128:(k + 1) * 128, :])
            wvt_sb.append(t_)
        bqk_sb = sb.tile([128, 4], f32, tag="bqk", name="bqk")
        nc.sync.dma_start(bqk_sb[:, :], bqk_d[:, :])
        bv_sb = sb.tile([128, CPG], f32, tag="bv", name="bv")
        nc.sync.dma_start(bv_sb[:, :], bv_d[:, :])
        mask_sb = sb.tile([128, 128], bf16, tag="mask", name="mask")
        nc.sync.dma_start(mask_sb[:, :], mask_d[:, :])
        wpt0_sb = sb.tile([128, C], bf16, tag="wpt0", name="wpt0")
        nc.sync.dma_start(wpt0_sb[:, :], wpt_d[0:128, :])
        wpt1_sb = sb.tile([64, C], bf16, tag="wpt1", name="wpt1")
        nc.sync.dma_start(wpt1_sb[:, :], wpt_d[128:CPG, :])
        ones_sb = sb.tile([1, 64], bf16, tag="ones", name="ones")
        nc.vector.memset(ones_sb[:, :], 1.0)

        # ---- QKV projection (q,k transposed; v natural) ----
        qt0 = sb.tile([128, T], bf16, tag="qt0", name="qt0")
        qt1 = sb.tile([64, T], bf16, tag="qt1", name="qt1")
        kt0 = sb.tile([128, T], bf16, tag="kt0", name="kt0")
        kt1 = sb.tile([64, T], bf16, tag="kt1", name="kt1")
        mtiles = [(qt0, 128, 0), (qt1, 64, 128), (kt0, 128, 192), (kt1, 64, 320)]
        for m, (dst, rows, c0) in enumerate(mtiles):
            for c in range(0, T, 1024):
                pq = mm_tile(f"pq{m}_{c}")
                for b0 in (0, 512):
                    for k in range(KT):
                        nc.tensor.matmul(
                            pq[0:rows, b0:b0 + 512],
                            lhsT=wqkt_sb[k][:, c0:c0 + rows],
                            rhs=xt_sb[k][:, c + b0:c + b0 + 512],
                            start=(k == 0), stop=(k == KT - 1),
                        )
                nc.vector.tensor_scalar_add(
                    dst[:, c:c + 1024], pq[0:rows, :], bqk_sb[0:rows, m:m + 1])

        v_sb = []
        for t in range(NT):
            pv = mm_tile(f"pv{t}")
            for k in range(KT):
                nc.tensor.matmul(
                    pv[:, 0:CPG],
                    lhsT=xt_sb[k][:, t * 128:(t + 1) * 128],
                    rhs=wvt_sb[k][:, :],
                    start=(k == 0), stop=(k == KT - 1),
                )
            vt = sb.tile([128, HPG * 65], bf16, tag=f"v{t}", name=f"v{t}")
            vt3 = vt.rearrange("p (h u) -> p h u", u=65)
            nc.vector.memset(vt3[:, :, 64:65], 1.0)
            nc.vector.tensor_add(
                vt3[:, :, 0:64],
                pv[:, 0:CPG].rearrange("p (h d) -> p h d", d=64),
                bv_sb[:, :].rearrange("p (h d) -> p h d", d=64),
            )
            v_sb.append(vt)

        # ---- attention: heads sequential, each in two tq-passes of 1024 ----
        q_slices = [(qt0, 0), (qt0, 64), (qt1, 0)]
        k_slices = [(kt0, 0), (kt0, 64), (kt1, 0)]
        pt0 = sb.tile([128, T], bf16, tag="pt0", name="pt0")
        pt1 = sb.tile([64, T], bf16, tag="pt1", name="pt1")
        p_slices = [(pt0, 0), (pt0, 64), (pt1, 0)]

        def do_pass(h, p):
            qa, qo = q_slices[h]
            ka, ko = k_slices[h]
            qv = qa[qo:qo + 64, :]
            kv = ka[ko:ko + 64, :]
            pdst, po = p_slices[h]
            base = p * PW
            i_max = (base + PW) // 128
            ot = ps.tile([65, PW], f32, tag="ot", bufs=1, name=f"ot{h}_{p}")
            ex_tiles = [None] * i_max

            def emit_s(i):
                lo = max(i * 128, base)
                ex = sb.tile([128, PW], bf16, tag="ex", bufs=4,
                             name=f"ex{h}_{p}_{i}")
                sp = mm_tile(f"sp{h}_{p}_{i}")
                for b0 in (0, 512):
                    cs = max(lo, base + b0)
                    ce = base + b0 + 512
                    if cs >= ce:
                        continue
                    nc.tensor.matmul(
                        sp[:, cs - base:ce - base],
                        lhsT=kv[:, i * 128:(i + 1) * 128],
                        rhs=qv[:, cs:ce],
                        start=True, stop=True,
                    )
                nc.scalar.activation(ex[:, lo - base:PW], sp[:, lo - base:PW],
                                     AF.Exp, scale=SCALE)
                if lo == i * 128:
                    # causal mask of the diagonal 128x128 block (GPSIMD: DVE
                    # and ScalarE are the loaded engines)
                    r = i * 128 - base
                    nc.gpsimd.tensor_mul(
                        ex[:, r:r + 128], ex[:, r:r + 128], mask_sb[:, :])
                ex_tiles[i] = ex

            def norm_chunk(b0):
                # rowsum bank complete: broadcast, reciprocal, scale, emit
                rsb = sb.tile([1, 512], bf16, tag="rsb", bufs=2,
                              name=f"rsb{h}_{p}_{b0}")
                nc.vector.tensor_copy(rsb[:, :], ot[64:65, b0:b0 + 512])
                bs = mm_tile(f"bs{h}_{p}_{b0}")
                nc.tensor.matmul(bs[0:64, 0:512], lhsT=ones_sb[:, :],
                                 rhs=rsb[:, :], start=True, stop=True)
                rb = sb.tile([64, 512], f32, tag="rb", bufs=2,
                             name=f"rb{h}_{p}_{b0}")
                nc.vector.reciprocal_approx_fast(rb[:, :], bs[0:64, 0:512])
                nc.vector.tensor_mul(
                    pdst[po:po + 64, base + b0:base + b0 + 512],
                    ot[0:64, b0:b0 + 512], rb[:, :])

            def emit_o(i):
                lo = max(i * 128, base)
                ex = ex_tiles[i]
                for b0 in (0, 512):
                    cs = max(lo, base + b0)
                    ce = base + b0 + 512
                    if cs >= ce:
                        continue
                    last_i = min(i_max - 1, (base + b0) // 128 + 3)
                    nc.tensor.matmul(
                        ot[:, cs - base:ce - base],
                        lhsT=v_sb[i][:, h * 65:(h + 1) * 65],
                        rhs=ex[:, cs - base:ce - base],
                        start=(i == 0), stop=(i == last_i),
                    )
                    if i == last_i:
                        norm_chunk(b0)

            # software pipeline: S runs 2 iterations ahead of O so the
            # ScalarE exp latency never stalls the PE.
            for i in range(i_max):
                emit_s(i)
                if i >= 2:
                    emit_o(i - 2)
            emit_o(i_max - 2)
            emit_o(i_max - 1)

        def emit_proj(trange):
            for t in trange:
                yp = mm_tile(f"yp{t}")
                for n0, nn in ((0, 512), (512, 256)):
                    nc.tensor.matmul(yp[:, n0:n0 + nn],
                                     lhsT=pt0[:, t * 128:(t + 1) * 128],
                                     rhs=wpt0_sb[:, n0:n0 + nn],
                                     start=True, stop=False)
                    nc.tensor.matmul(yp[:, n0:n0 + nn],
                                     lhsT=pt1[:, t * 128:(t + 1) * 128],
                                     rhs=wpt1_sb[:, n0:n0 + nn],
                                     start=False, stop=True)
                ysb = sb.tile([128, C], f32, tag=f"ysb{t % 4}", bufs=2,
                              name=f"ysb{t}")
                nc.vector.tensor_copy(ysb[:, 0:C], yp[:, 0:C])
                nc.sync.dma_start(y_d[t * 128:(t + 1) * 128, :], ysb[:, 0:C])

        for h in range(HPG):
            do_pass(h, 0)
            do_pass(h, 1)
        # proj for tq 0:1024 could start after all pass-0s; heads run
        # sequentially so just split the tail to overlap h2 pass 1 a bit.
        emit_proj(range(0, NT))

    nc.finalize()
    return nc


def _get_module():
    if "nc" not in _CACHE:
        _CACHE["nc"] = _build_module()
    return _CACHE["nc"]


def make_in_maps(x, w_attn, b_attn, w_proj):
    """Host-side sharding: per-core input dicts (8 cores)."""
    import ml_dtypes
    bf16 = ml_dtypes.bfloat16
    x = np.asarray(x, dtype=np.float32)
    w_attn = np.asarray(w_attn, dtype=np.float32)
    b_attn = np.asarray(b_attn, dtype=np.float32)
    w_proj = np.asarray(w_proj, dtype=np.float32)

    xts = [np.ascontiguousarray(x[b].T).astype(bf16) for b in range(B)]
    mask = np.triu(np.ones((128, 128), np.float32)).astype(bf16)

    in_maps = []
    for c in range(8):
        b = c // G
        hg = c % G
        sl = slice(CPG * hg, CPG * (hg + 1))
        wq = w_attn[0:C][sl]
        wk = w_attn[C:2 * C][sl]
        wv = w_attn[2 * C:3 * C][sl]
        wqkt = np.ascontiguousarray(
            np.concatenate([wq, wk], axis=0).T).astype(bf16)      # [768, 384]
        wvt = np.ascontiguousarray(wv.T).astype(bf16)             # [768, 192]
        bq = b_attn[0:C][sl]
        bk = b_attn[C:2 * C][sl]
        bv = b_attn[2 * C:3 * C][sl]
        bqk = np.zeros((128, 4), np.float32)
        bqk[:, 0] = bq[0:128]
        bqk[0:64, 1] = bq[128:192]
        bqk[:, 2] = bk[0:128]
        bqk[0:64, 3] = bk[128:192]
        bvb = np.ascontiguousarray(
            np.broadcast_to(bv, (128, CPG))).astype(np.float32)   # [128, 192]
        wpt = np.ascontiguousarray(w_proj[:, sl].T).astype(bf16)  # [192, 768]
        in_maps.append({
            "xt": xts[b],
            "wqkt": wqkt,
            "wvt": wvt,
            "bqk": bqk,
            "bv": bvb,
            "wpt": wpt,
            "mask": mask,
        })
    return in_maps


def gather(results, b_proj):
    """Sum the 4 head-group partials per batch, add bias."""
    b_proj = np.asarray(b_proj, dtype=np.float32)
    y = np.zeros((B, T, C), np.float32)
    for c in range(8):
        y[c // G] += np.asarray(results[c]["y"], dtype=np.float32)
    y += b_proj
    return y


def run(x, w_attn, b_attn, w_proj, b_proj, trace=False, **kw):
    from concourse.bass_utils import run_bass_kernel_spmd
    nc = _get_module()
    in_maps = make_in_maps(x, w_attn, b_attn, w_proj)
    res = run_bass_kernel_spmd(nc, in_maps, list(range(8)), trace=trace, **kw)
    return gather(res.results, b_proj), res


def kernel(x, w_attn, b_attn, w_proj, b_proj):
    y, _ = run(x, w_attn, b_attn, w_proj, b_proj)
    return y
